# revision 1
# baseline (speedup 1.0000x reference)
"""Adaptive softmax (head + 2 factorized tails) on 8 TRN2 NeuronCores.

Strategy: pure data-parallel over the 4096 tokens (512/core, weights
replicated). Host-side prep: weights pre-transposed to [K, V] layout and
cast to bf16 so the TensorEngine consumes them directly; head bias folded
in as an extra contraction row against a ones-row appended to x.

Per core, per cluster: TensorE accumulates 512-col logit chunks into
[128,2048] PSUM supers (K-contiguous, 4 token-tiles deep); DVE (3/4) and
ACT (1/4) evacuate PSUM into resident bf16 logit segments ([128,4096]
tiles in one 16-slot ring shared by all clusters — segment release is
what lets the next cluster's compute overlap the previous cluster's
output drain); ACT computes Exp per segment with accum_out giving the
row-sum Z partials; the output pass out = logit - (lse_tail + lse_head -
head_cluster_logit) is emitted as deferred per-segment units (DVE
tensor_scalar_sub / ACT Identity+bias) interleaved between the next
cluster's supers, staged as bf16 and DMA'd out (host upcasts to f32).
tail1 runs as two 2-token-tile groups (its logits don't fit SBUF in one
pass); the final group's staging borrows freed segment slots.
"""

import sys
import types

for _p in ("/opt/trn_rl_repo",):
    if _p not in sys.path:
        sys.path.append(_p)

import numpy as np
import ml_dtypes

N, H = 4096, 1024
CUT0, CUT1, VOCAB = 4000, 20000, 50257
HEAD_OUT = CUT0 + 2            # 4002
HEAD_PAD = 4096                # padded head cols (pad logit = -30 via bias row)
P0, P1 = 1024, 256
NCORES = 8
T = N // NCORES                # 512 tokens per core
TT = T // 128                  # 4 token tiles
KX = 9                         # x k-tiles: 8 real + 1 (ones row for head bias)
KAUG = KX * 128                # 1152

BF16 = ml_dtypes.bfloat16

_COMPILED = {}


def _ceil_div(a, b):
    return -(-a // b)


def _chunks(total, width):
    return [(s, min(width, total - s)) for s in range(0, total, width)]


def _build():
    import concourse.tile as tile
    from concourse import bacc, mybir

    F32 = mybir.dt.float32
    BF = mybir.dt.bfloat16
    Exp = mybir.ActivationFunctionType.Exp
    Ln = mybir.ActivationFunctionType.Ln
    AX = mybir.AxisListType.X

    nc = bacc.Bacc("TRN2", target_bir_lowering=False, debug=False,
                   num_devices=NCORES)

    xT_d = nc.dram_tensor("xT", [KAUG, T], BF, kind="ExternalInput").ap()
    hwT_d = nc.dram_tensor("hwT", [KAUG, HEAD_PAD], BF, kind="ExternalInput").ap()
    w01_d = nc.dram_tensor("w01T", [H, P0], BF, kind="ExternalInput").ap()
    w02_d = nc.dram_tensor("w02T", [P0, CUT1 - CUT0], BF, kind="ExternalInput").ap()
    w11_d = nc.dram_tensor("w11T", [H, P1], BF, kind="ExternalInput").ap()
    w12_d = nc.dram_tensor("w12T", [P1, VOCAB - CUT1], BF, kind="ExternalInput").ap()
    out_d = nc.dram_tensor("out", [T, VOCAB], BF, kind="ExternalOutput").ap()

    x_r = xT_d.rearrange("(k p) t -> p k t", p=128)        # [128, 9, 512]
    hw_r = hwT_d.rearrange("(k p) v -> p k v", p=128)      # [128, 9, 4096]
    w01_r = w01_d.rearrange("(k p) m -> p k m", p=128)     # [128, 8, 1024]
    w02_r = w02_d.rearrange("(k p) v -> p k v", p=128)     # [128, 8, 16000]
    w11_r = w11_d.rearrange("(k p) m -> p k m", p=128)     # [128, 8, 256]
    w12_r = w12_d.rearrange("(k p) v -> p k v", p=128)     # [128, 2, 30257]

    V0 = CUT1 - CUT0            # 16000
    V1 = VOCAB - CUT1           # 30257

    with tile.TileContext(nc, pool_alloc_mode="queue") as tc:
        with (
            tc.tile_pool(name="persist", bufs=1) as persist,
            tc.tile_pool(name="smalls", bufs=1) as smalls,
            tc.tile_pool(name="stage", bufs=2) as stage,
            tc.tile_pool(name="wload", bufs=4) as wload,
            tc.tile_pool(name="segs", bufs=16) as segpool,
            tc.tile_pool(name="psum", bufs=2, space="PSUM") as psum_pool,
        ):
            SEG = 4096

            # ---- persistent activations ----
            h0T_s = persist.tile([128, 8, T], BF, tag="h0T")
            h1T_s = persist.tile([128, 2, T], BF, tag="h1T")

            # ---- tiny per-row scalars ----
            zb = smalls.tile([128, 1], F32, tag="zb")       # zero bias for ACT
            nc.vector.memset(zb, 0.0)

            def sc(tag):
                return smalls.tile([128, 1], F32, tag=tag, name=tag)

            lse_h = [sc(f"lse_h{t}") for t in range(TT)]
            l40 = [sc(f"l40_{t}") for t in range(TT)]       # head logit col 4000
            l41 = [sc(f"l41_{t}") for t in range(TT)]       # head logit col 4001
            d0 = [sc(f"d0_{t}") for t in range(TT)]
            d1 = [sc(f"d1_{t}") for t in range(TT)]
            nd1 = [sc(f"nd1_{t}") for t in range(TT)]   # -d1 (ACT bias form)
            nd0 = [sc(f"nd0_{t}") for t in range(TT)]
            nlse_h = [sc(f"nlse_h{t}") for t in range(TT)]
            Ztmp = [sc(f"Ztmp{t}") for t in range(TT)]
            lse_t = [sc(f"lse_t{t}") for t in range(TT)]
            zh_p = [smalls.tile([128, 4], F32, tag=f"zh_p{t}", name=f"zh_p{t}")
                    for t in range(TT)]
            z0_p = [smalls.tile([128, 8], F32, tag=f"z0_p{t}", name=f"z0_p{t}")
                    for t in range(TT)]
            z1_p = [smalls.tile([128, 16], F32, tag=f"z1_p{t}", name=f"z1_p{t}")
                    for t in range(TT)]

            # ---- phase 0: h0T = w01T.T @ xT, h1T = w11T.T @ xT (bf16) ----
            xtp = tc.alloc_tile_pool(name="xtp", bufs=1)
            xT_s = xtp.tile([128, KX, T], BF, tag="xT", name="xT")
            nc.sync.dma_start(out=xT_s, in_=x_r)
            w11_s = wload.tile([128, 8, P1], BF, tag="wload", name="w11")
            nc.sync.dma_start(out=w11_s, in_=w11_r)

            for quad in range(2):   # h0T: 8 m-tiles, 4 per psum tile
                w01_s = wload.tile([128, 8, 512], BF, tag="wload",
                                   name="w01h")
                nc.sync.dma_start(
                    out=w01_s, in_=w01_r[:, :, quad * 512:(quad + 1) * 512])
                ps = psum_pool.tile([128, 2048], F32, tag="ps", name="ps_h0")
                for q in range(4):
                    for k in range(8):
                        nc.tensor.matmul(
                            ps[:, q * 512:(q + 1) * 512],
                            lhsT=w01_s[:, k, q * 128:(q + 1) * 128],
                            rhs=xT_s[:, k, :],
                            start=(k == 0), stop=(k == 7),
                        )
                nc.vector.tensor_copy(
                    out=h0T_s[:, 4 * quad:4 * quad + 4, :], in_=ps[:, :])

            ps1 = psum_pool.tile([128, 2048], F32, tag="ps", name="ps_h1")
            for m in range(2):
                for k in range(8):
                    nc.tensor.matmul(
                        ps1[:, m * 512:(m + 1) * 512],
                        lhsT=w11_s[:, k, m * 128:(m + 1) * 128],
                        rhs=xT_s[:, k, :],
                        start=(k == 0), stop=(k == 7),
                    )
            nc.vector.tensor_copy(out=h1T_s[:, 0:2, :], in_=ps1[:, 0:1024])

            seg_tiles = {}

            def seg_of(cl, tt, si):
                key = (cl, tt, si)
                if key not in seg_tiles:
                    seg_tiles[key] = segpool.tile(
                        [128, SEG], BF, tag="seg", name=f"seg_{cl}_{tt}_{si}")
                return seg_tiles[key]

            # ---- generic cluster processor ----
            def run_cluster(name, wT_r, Vtot, Kt, lhsT_of, tts, zp, load_w,
                            super_w, wpool, drain=None, exp_scratch=None,
                            evac_mod=0):
                loads = _chunks(Vtot, load_w)
                supers = _chunks(Vtot, super_w)
                n_drain0 = len(drain) if drain else 0
                nsup = len(supers)
                ld_tiles = {}

                def load(li):
                    c0, w = loads[li]
                    t_ = wpool.tile([128, Kt, load_w], BF, tag="wload",
                                    name=f"w_{name}")
                    nc.sync.dma_start(out=t_[:, :, :w], in_=wT_r[:, :, c0:c0 + w])
                    return t_

                for si, (sc0, sw) in enumerate(supers):
                    li0 = sc0 // load_w
                    li1 = _ceil_div(sc0 + sw, load_w)
                    for li in range(li0, li1):
                        if li not in ld_tiles:
                            ld_tiles[li] = load(li)
                    for ti, tt in enumerate(tts):
                        ps = psum_pool.tile([128, 2048], F32, tag="ps",
                                            name=f"ps_{name}")
                        for (c0, cw) in _chunks(sw, min(512, load_w)):
                            vc0 = sc0 + c0
                            li = vc0 // load_w
                            off = vc0 - loads[li][0]
                            wt = ld_tiles[li]
                            for k in range(Kt):
                                nc.tensor.matmul(
                                    ps[:, c0:c0 + cw],
                                    lhsT=lhsT_of(k, tt),
                                    rhs=wt[:, k, off:off + cw],
                                    start=(k == 0), stop=(k == Kt - 1),
                                )
                        seg = seg_of(name, tt, sc0 // SEG)
                        so = sc0 % SEG
                        if evac_mod and (si * len(tts) + ti) % evac_mod == 0:
                            nc.scalar.copy(out=seg[:, so:so + sw],
                                           in_=ps[:, :sw])
                        else:
                            nc.vector.tensor_copy(out=seg[:, so:so + sw],
                                                  in_=ps[:, :sw])
                        if exp_scratch is None:
                            # per-super exp, in place on PSUM
                            nc.scalar.activation(
                                out=ps[:, :sw], in_=seg[:, so:so + sw],
                                func=Exp, bias=zb, scale=1.0,
                                accum_out=zp[tt][:, si:si + 1],
                            )
                        elif so + sw >= SEG or sc0 + sw >= Vtot:
                            # segment complete: one exp + Z-partial per seg
                            seg_w = so + sw
                            sidx = sc0 // SEG
                            extag = "st" if exp_scratch is stage else "ex"
                            ex = exp_scratch.tile(
                                [128, SEG], BF, tag=extag, name="ex")
                            nc.scalar.activation(
                                out=ex[:, :seg_w], in_=seg[:, :seg_w],
                                func=Exp, bias=zb, scale=1.0,
                                accum_out=zp[tt][:, sidx:sidx + 1],
                            )
                    for li in range(li0, li1):
                        if (li + 1) * load_w <= sc0 + sw:
                            ld_tiles.pop(li, None)
                    if n_drain0:
                        want = (n_drain0 * (si + 1)) // nsup - \
                               (n_drain0 * si) // nsup
                        for _ in range(want):
                            if drain:
                                drain.pop(0)()

            def d_units(cl, tt, d_ap, nd_ap, out_c0, out_w, units,
                        act_frac=0.5, st_pool=None, st_skip=0):
                """Append deferred output units: out = logit - d, per seg.
                Split between DVE (tensor_scalar_sub with d) and ACT
                (Identity with bias -d) at the given ACT fraction."""
                r0 = tt * 128
                for idx, (c0, cw) in enumerate(_chunks(out_w, SEG)):
                    on_act = (int((idx + 1) * act_frac) != int(idx * act_frac))
                    def emit(c0=c0, cw=cw, idx=idx, on_act=on_act,
                             tt=tt, cl=cl):
                        seg = seg_of(cl, tt, c0 // SEG)
                        use_sp = st_pool is not None and idx >= st_skip
                        pool = st_pool if use_sp else stage
                        tg = "seg" if use_sp else "st"
                        st = pool.tile([128, SEG], BF, tag=tg, name="st")
                        if not on_act:
                            nc.vector.tensor_scalar_sub(
                                st[:, :cw], seg[:, :cw], d_ap)
                        else:
                            nc.scalar.add(st[:, :cw], seg[:, :cw], nd_ap)
                        nc.sync.dma_start(
                            out=out_d[r0:r0 + 128,
                                      out_c0 + c0:out_c0 + c0 + cw],
                            in_=st[:, :cw],
                        )
                    units.append(emit)

            # ---- phase 1: head ----
            run_cluster(
                "h", hw_r, HEAD_PAD, KX,
                lambda k, tt: xT_s[:, k, tt * 128:(tt + 1) * 128],
                list(range(TT)), zh_p, 512, 1024, wload,
                exp_scratch=stage,
            )
            xtp.release()
            scratch = tc.alloc_tile_pool(name="scratch", bufs=1)
            pend = []
            for tt in range(TT):
                nc.vector.reduce_sum(out=Ztmp[tt], in_=zh_p[tt][:, 0:1],
                                     axis=AX)
                nc.scalar.activation(out=lse_h[tt], in_=Ztmp[tt],
                                     func=Ln, bias=zb, scale=1.0)
                nc.vector.tensor_sub(nlse_h[tt], zb, lse_h[tt])
                sg0 = seg_of("h", tt, 0)
                nc.vector.tensor_copy(out=l40[tt], in_=sg0[:, 4000:4001])
                nc.vector.tensor_copy(out=l41[tt], in_=sg0[:, 4001:4002])
                d_units("h", tt, lse_h[tt], nlse_h[tt], 0, CUT0, pend)

            # ---- phase 2: tail 0 (16000 cols, K=8 over h0T) ----
            run_cluster(
                "t0", w02_r, V0, 8,
                lambda k, tt: h0T_s[:, k, tt * 128:(tt + 1) * 128],
                list(range(TT)), z0_p, 512, 2048, wload, drain=pend,
                exp_scratch=scratch,
            )
            pend = []
            for tt in range(TT):
                nc.vector.reduce_sum(out=Ztmp[tt], in_=z0_p[tt][:, 0:4],
                                     axis=AX)
                nc.scalar.activation(out=lse_t[tt], in_=Ztmp[tt],
                                     func=Ln, bias=zb, scale=1.0)
                nc.vector.tensor_add(d0[tt], lse_t[tt], lse_h[tt])
                nc.vector.tensor_sub(d0[tt], d0[tt], l40[tt])
                nc.vector.tensor_sub(nd0[tt], zb, d0[tt])
                d_units("t0", tt, d0[tt], nd0[tt], CUT0, V0, pend,
                        act_frac=0.25)

            # ---- phase 3: tail 1 (30257 cols, K=2 over h1T), 2 tt-groups ----
            for grp in range(2):
                tts = [2 * grp, 2 * grp + 1]
                run_cluster(
                    f"t1g{grp}", w12_r, V1, 2,
                    lambda k, tt: h1T_s[:, k, tt * 128:(tt + 1) * 128],
                    tts, z1_p, 2048, 2048, wload, drain=pend,
                    exp_scratch=scratch, evac_mod=4,
                )
                pend = []
                for tt in tts:
                    nc.vector.reduce_sum(out=Ztmp[tt],
                                         in_=z1_p[tt][:, 0:8], axis=AX)
                    nc.scalar.activation(out=lse_t[tt], in_=Ztmp[tt],
                                         func=Ln, bias=zb, scale=1.0)
                    nc.vector.tensor_add(d1[tt], lse_t[tt], lse_h[tt])
                    nc.vector.tensor_sub(d1[tt], d1[tt], l41[tt])
                    nc.vector.tensor_sub(nd1[tt], zb, d1[tt])
                    d_units(f"t1g{grp}", tt, d1[tt], nd1[tt], CUT1, V1, pend,
                        act_frac=0.0 if grp == 0 else 0.5,
                        st_pool=segpool if grp == 1 else None,
                        st_skip=2 if tt == tts[0] else 0)
            for u in pend:
                u()
            scratch.release()

    nc.compile()
    return nc


def _get_nc():
    if "nc" not in _COMPILED:
        _COMPILED["nc"] = _build()
    return _COMPILED["nc"]


def _prep_inputs(x, head_w, head_b, t0_w1, t0_w2, t1_w1, t1_w2):
    f32 = np.float32

    hwT = np.zeros((KAUG, HEAD_PAD), dtype=f32)
    hwT[:H, :HEAD_OUT] = np.asarray(head_w, f32).T
    hwT[H, :HEAD_OUT] = np.asarray(head_b, f32)
    hwT[H, HEAD_OUT:] = -30.0
    hwT = hwT.astype(BF16)

    w01T = np.ascontiguousarray(np.asarray(t0_w1, f32).T).astype(BF16)
    w02T = np.ascontiguousarray(np.asarray(t0_w2, f32).T).astype(BF16)
    w11T = np.ascontiguousarray(np.asarray(t1_w1, f32).T).astype(BF16)
    w12T = np.ascontiguousarray(np.asarray(t1_w2, f32).T).astype(BF16)

    in_maps = []
    for c in range(NCORES):
        xs = np.asarray(x[c * T:(c + 1) * T], f32)
        xT = np.zeros((KAUG, T), dtype=f32)
        xT[:H] = xs.T
        xT[H] = 1.0
        in_maps.append({
            "xT": xT.astype(BF16),
            "hwT": hwT,
            "w01T": w01T,
            "w02T": w02T,
            "w11T": w11T,
            "w12T": w12T,
        })
    return in_maps


def run(trace=False, **inputs):
    from concourse.bass_utils import run_bass_kernel_spmd

    if trace:
        try:
            if "antenv.axon_hooks" not in sys.modules:
                if "/root/.axon_site" not in sys.path:
                    sys.path.append("/root/.axon_site")
                import trn_agent_boot.trn_boot as tb
                hook = tb._ntff_profile_via_ctypes("/opt/axon/libaxon_pjrt.so")
                mod = types.ModuleType("antenv.axon_hooks")
                mod.get_axon_ntff_profile_hook = lambda: hook
                sys.modules["antenv.axon_hooks"] = mod
        except Exception:
            trace = False

    nc = _get_nc()
    in_maps = _prep_inputs(**inputs)
    last_err = None
    for attempt in range(3):
        try:
            res = run_bass_kernel_spmd(nc, in_maps,
                                       core_ids=list(range(NCORES)),
                                       trace=trace)
            break
        except Exception as e:  # transient NRT device errors: retry
            last_err = e
    else:
        raise last_err
    out = np.concatenate([res.results[i]["out"] for i in range(NCORES)],
                         axis=0).astype(np.float32)
    return out, res


def kernel(**inputs):
    out, _ = run(trace=False, **inputs)
    return out


if __name__ == "__main__":
    rng = np.random.default_rng(0)
    ins = {
        "x": rng.standard_normal((N, H), dtype=np.float32),
        "head_w": (rng.standard_normal((HEAD_OUT, H), dtype=np.float32) / 32),
        "head_b": (rng.standard_normal(HEAD_OUT).astype(np.float32) * 0.01),
        "t0_w1": (rng.standard_normal((P0, H), dtype=np.float32) / 32),
        "t0_w2": (rng.standard_normal((CUT1 - CUT0, P0), dtype=np.float32) / 32),
        "t1_w1": (rng.standard_normal((P1, H), dtype=np.float32) / 32),
        "t1_w2": (rng.standard_normal((VOCAB - CUT1, P1), dtype=np.float32) / 16),
    }
    out, res = run(trace=False, **ins)
    print("out", out.shape, out.dtype)



# revision 12
# speedup vs baseline: 1.1072x; 1.1072x over previous
"""Adaptive softmax (head + 2 factorized tails) on 8 TRN2 NeuronCores.

v2: fp8 everywhere. Data-parallel over tokens (512/core), weights
replicated, cast to fp8-e4m3 host-side. All matmuls run DoubleRow
(2 k-tiles per instruction, 2x bf16 FLOP rate), with g-major/chunk-minor
accumulation so LDWEIGHTS amortizes over 4 matmuls. Logit segments are
staged in fp8-e3m4 SBUF tiles; PSUM evacuation is split between DVE
(tensor_copy / bias-adding scalar_tensor_tensor) and ACT (Identity
activation) per a measured-rate balance; Exp+row-sum partials run on ACT
reading PSUM directly. Output units compute (logit - d + C) into
fp8-e4m3 staging tiles (DVE tensor_scalar at the 2x fp8 rate, plus ACT
Identity+bias for the tail-most units) and DMA out; the host decodes
with float32(out) - C. Head bias (+ -30 padding) is folded in as a
free-dim bias tile added during head evacuation. Cluster order:
proj -> head -> t0 -> t1(tt0-2) -> t1(tt3); each cluster's output units
drain interleaved into the next cluster's compute so the serial tail is
only tt3's vocab slice.
"""

import sys
import types

for _p in ("/opt/trn_rl_repo",):
    if _p not in sys.path:
        sys.path.append(_p)

import numpy as np
import ml_dtypes

N, H = 4096, 1024
CUT0, CUT1, VOCAB = 4000, 20000, 50257
HEAD_OUT = CUT0 + 2            # 4002
HEAD_PAD = 4096                # padded head cols (pad logit = -30 via bias)
P0, P1 = 1024, 256
V0 = CUT1 - CUT0               # 16000
V1 = VOCAB - CUT1              # 30257
V1P = 30272                    # padded (mult of 64; pad weight cols = 0)
NCORES = 8
T = N // NCORES                # 512 tokens per core
TT = T // 128                  # 4 token tiles
C_OFF = 18.9375                # output offset: device stores out + C_OFF

E4 = ml_dtypes.float8_e4m3
E3 = ml_dtypes.float8_e3m4
BF16 = ml_dtypes.bfloat16

_COMPILED = {}


def _chunks(total, width):
    return [(s, min(width, total - s)) for s in range(0, total, width)]


def _build():
    import concourse.tile as tile
    from concourse import bacc, mybir
    from concourse.alu_op_type import AluOpType

    F32 = mybir.dt.float32
    BF = mybir.dt.bfloat16
    F8E4 = mybir.dt.float8e4
    F8E3 = mybir.dt.float8e3
    Exp = mybir.ActivationFunctionType.Exp
    Ln = mybir.ActivationFunctionType.Ln
    DR = mybir.MatmulPerfMode.DoubleRow
    AX = mybir.AxisListType.X

    nc = bacc.Bacc("TRN2", target_bir_lowering=False, debug=False,
                   num_devices=NCORES)

    xT_d = nc.dram_tensor("xT", [H, T], F8E4, kind="ExternalInput").ap()
    hwT_d = nc.dram_tensor("hwT", [H, HEAD_PAD], F8E4, kind="ExternalInput").ap()
    hb_d = nc.dram_tensor("hb", [128, HEAD_PAD], BF, kind="ExternalInput").ap()
    w01_d = nc.dram_tensor("w01T", [H, P0], F8E4, kind="ExternalInput").ap()
    w02_d = nc.dram_tensor("w02T", [P0, V0], F8E4, kind="ExternalInput").ap()
    w11_d = nc.dram_tensor("w11T", [H, P1], F8E4, kind="ExternalInput").ap()
    w12_d = nc.dram_tensor("w12T", [P1, V1P], F8E4, kind="ExternalInput").ap()
    out_d = nc.dram_tensor("out", [T, VOCAB], F8E4, kind="ExternalOutput").ap()

    x_r = xT_d.rearrange("(k p) t -> p k t", p=128)        # [128, 8, 512]
    hw_r = hwT_d.rearrange("(k p) v -> p k v", p=128)      # [128, 8, 4096]
    w01_r = w01_d.rearrange("(k p) m -> p k m", p=128)     # [128, 8, 1024]
    w02_r = w02_d.rearrange("(k p) v -> p k v", p=128)     # [128, 8, 16000]
    w11_r = w11_d.rearrange("(k p) m -> p k m", p=128)     # [128, 8, 256]
    w12_r = w12_d.rearrange("(k p) v -> p k v", p=128)     # [128, 2, 30272]

    SUPW = 2048
    sup_h = _chunks(HEAD_PAD, 1024)        # 4 supers of 1024
    sup_t0 = _chunks(V0, SUPW)             # 8 (last 1664)
    sup_t1 = _chunks(V1P, SUPW)            # 15 (last 1600)

    with tile.TileContext(nc, pool_alloc_mode="queue") as tc:
        # pool stack: long-lived below, transients on top (LIFO releases)
        smalls = tc.alloc_tile_pool(name="smalls", bufs=1)
        stage = tc.alloc_tile_pool(name="stage", bufs=2)
        scr = tc.alloc_tile_pool(name="scr", bufs=2)
        psum_pool = tc.alloc_tile_pool(name="psum", bufs=2, space="PSUM")
        persist = tc.alloc_tile_pool(name="persist", bufs=1)
        h0T_s = persist.tile([128, 8, T], F8E4, tag="h0T")
        h1T_s = persist.tile([128, 2, T], F8E4, tag="h1T")
        t1segs = {}
        t1segpA = tc.alloc_tile_pool(name="t1segsA", bufs=1)
        for t in (0, 1):
            t1segs[t] = t1segpA.tile([128, V1P], F8E3, tag=f"t1seg{t}",
                                     name=f"t1seg{t}")
        t1wp = tc.alloc_tile_pool(name="t1wp", bufs=2)
        t0segp = tc.alloc_tile_pool(name="t0segs", bufs=1)
        t0segs = [t0segp.tile([128, V0], F8E3, tag=f"t0seg{t}",
                              name=f"t0seg{t}")
                  for t in range(TT)]
        hsegp = tc.alloc_tile_pool(name="hsegs", bufs=1)
        hsegs = [hsegp.tile([128, HEAD_PAD], F8E3, tag=f"hseg{t}",
                            name=f"hseg{t}")
                 for t in range(TT)]

        # ---- tiny per-row scalars ----
        def sc(tag, w=1):
            return smalls.tile([128, w], F32, tag=tag, name=tag)

        zb = sc("zb")
        nc.vector.memset(zb, 0.0)
        lse_h = [sc(f"lse_h{t}") for t in range(TT)]
        lse_t = [sc(f"lse_t{t}") for t in range(TT)]
        Ztmp = [sc(f"Ztmp{t}") for t in range(TT)]
        l4x = [sc(f"l4x{t}", 2) for t in range(TT)]        # cols 4000,4001
        dh = [sc(f"dh{t}") for t in range(TT)]             # lse_h - C
        ndh = [sc(f"ndh{t}") for t in range(TT)]
        d0 = [sc(f"d0_{t}") for t in range(TT)]
        nd0 = [sc(f"nd0_{t}") for t in range(TT)]
        d1 = [sc(f"d1_{t}") for t in range(TT)]
        nd1 = [sc(f"nd1_{t}") for t in range(TT)]
        zh_p = [sc(f"zh_p{t}", len(sup_h)) for t in range(TT)]
        z0_p = [sc(f"z0_p{t}", len(sup_t0)) for t in range(TT)]
        z1_p = [sc(f"z1_p{t}", len(sup_t1)) for t in range(TT)]

        # ---- transient pools for proj + head ----
        xtp = tc.alloc_tile_pool(name="xtp", bufs=1)
        xT_s = xtp.tile([128, 8, T], F8E4, tag="xT", name="xT")
        nc.sync.dma_start(out=xT_s, in_=x_r)
        projw = tc.alloc_tile_pool(name="projw", bufs=1)
        w01_s = projw.tile([128, 8, P0], F8E4, tag="w01", name="w01")
        w11_s = projw.tile([128, 8, P1], F8E4, tag="w11", name="w11")
        nc.sync.dma_start(out=w01_s, in_=w01_r)
        nc.sync.dma_start(out=w11_s, in_=w11_r)
        hbp = tc.alloc_tile_pool(name="hbp", bufs=1)
        hb_s = hbp.tile([128, HEAD_PAD], BF, tag="hb", name="hb")
        nc.sync.dma_start(out=hb_s, in_=hb_d)
        hwp = tc.alloc_tile_pool(name="hwp", bufs=2)

        # ---- phase 0: proj matmuls ----
        for half in range(2):   # h0T: 8 m-tiles, 4 per psum super
            ps = psum_pool.tile([128, 2048], F32, tag="ps", name=f"psh0{half}")
            for g in range(4):
                for m in range(4):
                    mm = half * 4 + m
                    nc.tensor.matmul(
                        ps[:, m * 512:(m + 1) * 512],
                        lhsT=w01_s[:, 2 * g:2 * g + 2, mm * 128:(mm + 1) * 128],
                        rhs=xT_s[:, 2 * g:2 * g + 2, :],
                        start=(g == 0), stop=(g == 3),
                        perf_mode=DR,
                    )
            nc.vector.tensor_copy(out=h0T_s[:, 4 * half:4 * half + 4, :],
                                  in_=ps)
        ps1 = psum_pool.tile([128, 2048], F32, tag="ps", name="ps_h1")
        for g in range(4):
            for m in range(2):
                nc.tensor.matmul(
                    ps1[:, m * 512:(m + 1) * 512],
                    lhsT=w11_s[:, 2 * g:2 * g + 2, m * 128:(m + 1) * 128],
                    rhs=xT_s[:, 2 * g:2 * g + 2, :],
                    start=(g == 0), stop=(g == 3),
                    perf_mode=DR,
                )
        nc.vector.tensor_copy(out=h1T_s, in_=ps1[:, 0:1024])

        pend = []   # deferred output-unit emitters

        def drain(n):
            for _ in range(min(n, len(pend))):
                pend.pop(0)()

        def run_cluster(name, sups, tts, wloader, lhsT_of, Kg, segs, zp,
                        evac_act, bias=None, drain_per_unit=0.0,
                        l4cap=None):
            acc = [0.0]
            for si, (c0, w) in enumerate(sups):
                wt_of = wloader(si)
                for tt in tts:
                    ps = psum_pool.tile([128, SUPW], F32, tag="ps",
                                        name=f"ps_{name}")
                    cks = _chunks(w, 512)
                    for g in range(Kg):
                        wt, off = wt_of(g)
                        for (cc, cw) in cks:
                            nc.tensor.matmul(
                                ps[:, cc:cc + cw],
                                lhsT=lhsT_of(g, tt),
                                rhs=wt[:, :, off + cc:off + cc + cw],
                                start=(g == 0), stop=(g == Kg - 1),
                                perf_mode=DR,
                            )
                    seg = segs[tt]
                    if l4cap is not None and si == l4cap[0]:
                        nc.vector.scalar_tensor_tensor(
                            out=l4x[tt], in0=ps[:, l4cap[1]:l4cap[1] + 2],
                            scalar=1.0, in1=hb_s[:, 4000:4002],
                            op0=AluOpType.mult, op1=AluOpType.add)
                    ex = scr.tile([128, SUPW], F8E4, tag="ex", name="ex")
                    if bias is not None:
                        # bias-adding evac on DVE; exp reads the seg so Z
                        # includes the bias
                        nc.vector.scalar_tensor_tensor(
                            out=seg[:, c0:c0 + w], in0=ps[:, :w], scalar=1.0,
                            in1=bias[:, c0:c0 + w],
                            op0=AluOpType.mult, op1=AluOpType.add)
                        nc.scalar.activation(
                            out=ex[:, :w], in_=seg[:, c0:c0 + w], func=Exp,
                            bias=zb, scale=1.0,
                            accum_out=zp[tt][:, si:si + 1])
                    else:
                        if evac_act(si, tt):
                            nc.scalar.copy(out=seg[:, c0:c0 + w],
                                           in_=ps[:, :w])
                        else:
                            nc.vector.tensor_copy(out=seg[:, c0:c0 + w],
                                                  in_=ps[:, :w])
                        nc.scalar.activation(
                            out=ex[:, :w], in_=ps[:, :w], func=Exp,
                            bias=zb, scale=1.0,
                            accum_out=zp[tt][:, si:si + 1])
                    acc[0] += drain_per_unit
                    k = int(acc[0])
                    if k:
                        acc[0] -= k
                        drain(k)

        def finish_rows(tts, zp, nsup, lse_out, d_out, nd_out, extra):
            for tt in tts:
                nc.vector.reduce_sum(out=Ztmp[tt], in_=zp[tt][:, 0:nsup],
                                     axis=AX)
                nc.scalar.activation(out=lse_out[tt], in_=Ztmp[tt],
                                     func=Ln, bias=zb, scale=1.0)
                if extra is None:
                    nc.vector.tensor_scalar_sub(d_out[tt], lse_out[tt], C_OFF)
                else:
                    ex_ap, l4col = extra
                    nc.vector.tensor_add(d_out[tt], lse_out[tt], ex_ap[tt])
                    nc.vector.scalar_tensor_tensor(
                        out=d_out[tt], in0=d_out[tt], scalar=C_OFF,
                        in1=l4x[tt][:, l4col:l4col + 1],
                        op0=AluOpType.subtract, op1=AluOpType.subtract)
                nc.vector.tensor_sub(nd_out[tt], zb, d_out[tt])

        def emit_units(segs, tt, d_ap, nd_ap, out_c0, width, on_act):
            r0 = tt * 128
            for ui, (c0, cw) in enumerate(_chunks(width, 4096)):
                def emit(c0=c0, cw=cw, ui=ui, tt=tt):
                    seg = segs[tt]
                    st = stage.tile([128, 4096], F8E4, tag="st", name="st")
                    if on_act(ui):
                        nc.scalar.add(st[:, :cw], seg[:, c0:c0 + cw], nd_ap)
                    else:
                        nc.vector.tensor_scalar_sub(
                            st[:, :cw], seg[:, c0:c0 + cw], d_ap)
                    nc.sync.dma_start(
                        out=out_d[r0:r0 + 128, out_c0 + c0:out_c0 + c0 + cw],
                        in_=st[:, :cw])
                pend.append(emit)

        # ================= HEAD (supers of 1024, 1 load each) =========
        hw_tiles = {}

        def hw_loader(si):
            if si not in hw_tiles:
                t_ = hwp.tile([128, 8, 1024], F8E4, tag="hw", name=f"hw{si}")
                nc.sync.dma_start(
                    out=t_, in_=hw_r[:, :, si * 1024:(si + 1) * 1024])
                hw_tiles[si] = t_
            tile_ = hw_tiles[si]
            return lambda g: (tile_[:, 2 * g:2 * g + 2, :], 0)

        run_cluster("h", sup_h, range(TT), hw_loader,
                    lambda g, tt: xT_s[:, 2 * g:2 * g + 2,
                                       tt * 128:(tt + 1) * 128],
                    4, hsegs, zh_p, lambda si, tt: False, bias=hb_s,
                    l4cap=(3, 928))
        finish_rows(range(TT), zh_p, len(sup_h), lse_h, dh, ndh, None)
        for tt in range(TT):
            emit_units(hsegs, tt, dh[tt], ndh[tt], 0, CUT0,
                       lambda ui: False)
        hwp.release()
        hbp.release()
        projw.release()
        xtp.release()

        # ================= T0 (k-split loads: 2 x [128,4,2048]) =======
        t0wp = tc.alloc_tile_pool(name="t0wp", bufs=3)
        t0_tiles = {}

        def t0_loader(si):
            if si not in t0_tiles:
                c0, w = sup_t0[si]
                ts = []
                for kh in range(2):
                    t_ = t0wp.tile([128, 4, SUPW], F8E4, tag="w02",
                                   name=f"w02_{si}_{kh}")
                    nc.sync.dma_start(
                        out=t_[:, :, :w],
                        in_=w02_r[:, 4 * kh:4 * kh + 4, c0:c0 + w])
                    ts.append(t_)
                t0_tiles[si] = ts
            ts = t0_tiles[si]
            return lambda g: (ts[g // 2][:, 2 * (g % 2):2 * (g % 2) + 2, :], 0)

        run_cluster("t0", sup_t0, range(TT), t0_loader,
                    lambda g, tt: h0T_s[:, 2 * g:2 * g + 2,
                                        tt * 128:(tt + 1) * 128],
                    4, t0segs, z0_p,
                    lambda si, tt: si % 4 == 0, drain_per_unit=0.2)
        finish_rows(range(TT), z0_p, len(sup_t0), lse_t, d0, nd0,
                    (lse_h, 0))
        for tt in range(TT):
            emit_units(t0segs, tt, d0[tt], nd0[tt], CUT0, V0,
                       lambda ui: False)
        t0wp.release()
        hsegp.release()

        # ================= T1 (group A: tt0,1; group B: tt2,3) =========
        def run_t1(grp, tts, dpu):
            t1_tiles = {}

            def t1_loader(si):
                if si not in t1_tiles:
                    c0, w = sup_t1[si]
                    t_ = t1wp.tile([128, 2, SUPW], F8E4, tag="w12",
                                   name=f"w12_{grp}_{si}")
                    nc.sync.dma_start(out=t_[:, :, :w],
                                      in_=w12_r[:, :, c0:c0 + w])
                    t1_tiles[si] = t_
                tile_ = t1_tiles[si]
                return lambda g: (tile_, 0)

            run_cluster(f"t1g{grp}", sup_t1, tts, t1_loader,
                        lambda g, tt: h1T_s[:, 0:2, tt * 128:(tt + 1) * 128],
                        1, t1segs, z1_p,
                        lambda si, tt: si % 2 == 0, drain_per_unit=dpu)
            finish_rows(tts, z1_p, len(sup_t1), lse_t, d1, nd1,
                        (lse_h, 1))
            for tt in tts:
                emit_units(t1segs, tt, d1[tt], nd1[tt], CUT1, V1,
                           (lambda ui: ui % 2 == 0) if grp == 1
                           else (lambda ui: False))

        # t0 outs (16 units) drain over gA's 30 steps
        run_t1(0, [0, 1], 0.55)
        t0segp.release()
        t1segpB = tc.alloc_tile_pool(name="t1segsB", bufs=1)
        for t in (2, 3):
            t1segs[t] = t1segpB.tile([128, V1P], F8E3, tag=f"t1seg{t}",
                                     name=f"t1seg{t}")
        # gA outs (16 units) drain over gB's 30 steps
        run_t1(1, [2, 3], 0.55)
        for u in pend:
            u()
        for p in (t1segpB, t1wp, t1segpA, persist, psum_pool, scr, stage,
                  smalls):
            p.release()

    nc.compile()
    return nc


def _get_nc():
    if "nc" not in _COMPILED:
        _COMPILED["nc"] = _build()
    return _COMPILED["nc"]


def _prep_inputs(x, head_w, head_b, t0_w1, t0_w2, t1_w1, t1_w2):
    f32 = np.float32

    hwT = np.zeros((H, HEAD_PAD), dtype=f32)
    hwT[:, :HEAD_OUT] = np.asarray(head_w, f32).T
    hb = np.full((HEAD_PAD,), -30.0, dtype=f32)
    hb[:HEAD_OUT] = np.asarray(head_b, f32)
    hbrep = np.ascontiguousarray(
        np.broadcast_to(hb, (128, HEAD_PAD))).astype(BF16)

    w12T = np.zeros((P1, V1P), dtype=f32)
    w12T[:, :V1] = np.asarray(t1_w2, f32).T

    ins_common = {
        "hwT": hwT.astype(E4),
        "hb": hbrep,
        "w01T": np.ascontiguousarray(np.asarray(t0_w1, f32).T).astype(E4),
        "w02T": np.ascontiguousarray(np.asarray(t0_w2, f32).T).astype(E4),
        "w11T": np.ascontiguousarray(np.asarray(t1_w1, f32).T).astype(E4),
        "w12T": w12T.astype(E4),
    }
    in_maps = []
    for c in range(NCORES):
        xs = np.asarray(x[c * T:(c + 1) * T], f32)
        m = {"xT": np.ascontiguousarray(xs.T).astype(E4)}
        m.update(ins_common)
        in_maps.append(m)
    return in_maps


def run(trace=False, **inputs):
    from concourse.bass_utils import run_bass_kernel_spmd

    if trace:
        try:
            if "antenv.axon_hooks" not in sys.modules:
                if "/root/.axon_site" not in sys.path:
                    sys.path.append("/root/.axon_site")
                import trn_agent_boot.trn_boot as tb
                hook = tb._ntff_profile_via_ctypes("/opt/axon/libaxon_pjrt.so")
                mod = types.ModuleType("antenv.axon_hooks")
                mod.get_axon_ntff_profile_hook = lambda: hook
                sys.modules["antenv.axon_hooks"] = mod
        except Exception:
            trace = False

    nc = _get_nc()
    in_maps = _prep_inputs(**inputs)
    last_err = None
    for attempt in range(3):
        try:
            res = run_bass_kernel_spmd(nc, in_maps,
                                       core_ids=list(range(NCORES)),
                                       trace=trace)
            break
        except Exception as e:  # transient NRT device errors: retry
            last_err = e
    else:
        raise last_err
    out = np.concatenate(
        [res.results[i]["out"].astype(np.float32) for i in range(NCORES)],
        axis=0)
    out -= C_OFF
    return out, res


def kernel(**inputs):
    out, _ = run(trace=False, **inputs)
    return out


if __name__ == "__main__":
    rng = np.random.default_rng(0)
    ins = {
        "x": rng.standard_normal((N, H), dtype=np.float32),
        "head_w": (rng.standard_normal((HEAD_OUT, H), dtype=np.float32) / 32),
        "head_b": (rng.standard_normal(HEAD_OUT).astype(np.float32) * 0.01),
        "t0_w1": (rng.standard_normal((P0, H), dtype=np.float32) / 32),
        "t0_w2": (rng.standard_normal((CUT1 - CUT0, P0), dtype=np.float32) / 32),
        "t1_w1": (rng.standard_normal((P1, H), dtype=np.float32) / 32),
        "t1_w2": (rng.standard_normal((VOCAB - CUT1, P1), dtype=np.float32) / 16),
    }
    out, res = run(trace=False, **ins)
    print("out", out.shape, out.dtype)


# revision 16
# speedup vs baseline: 1.2912x; 1.1661x over previous
"""Adaptive softmax (head + 2 factorized tails) on 8 TRN2 NeuronCores.

v2: fp8 everywhere. Data-parallel over tokens (512/core), weights
replicated, cast to fp8-e4m3 host-side. All matmuls run DoubleRow
(2 k-tiles per instruction, 2x bf16 FLOP rate), with g-major/chunk-minor
accumulation so LDWEIGHTS amortizes over 4 matmuls. Logit segments are
staged in fp8-e3m4 SBUF tiles; PSUM evacuation is split between DVE
(tensor_copy / bias-adding scalar_tensor_tensor) and ACT (Identity
activation) per a measured-rate balance; Exp+row-sum partials run on ACT
reading PSUM directly. Output units compute (logit - d + C) into
fp8-e4m3 staging tiles (DVE tensor_scalar at the 2x fp8 rate, plus ACT
Identity+bias for the tail-most units) and DMA out; the host decodes
with float32(out) - C. Head bias (+ -30 padding) is folded in as a
free-dim bias tile added during head evacuation. Cluster order:
proj -> head -> t0 -> t1(tt0-2) -> t1(tt3); each cluster's output units
drain interleaved into the next cluster's compute so the serial tail is
only tt3's vocab slice.
"""

import sys
import types

for _p in ("/opt/trn_rl_repo",):
    if _p not in sys.path:
        sys.path.append(_p)

import numpy as np
import ml_dtypes

N, H = 4096, 1024
CUT0, CUT1, VOCAB = 4000, 20000, 50257
HEAD_OUT = CUT0 + 2            # 4002
HEAD_PAD = 4096                # padded head cols (pad logit = -30 via bias)
P0, P1 = 1024, 256
V0 = CUT1 - CUT0               # 16000
V1 = VOCAB - CUT1              # 30257
V1P = 30272                    # padded (mult of 64; pad weight cols = 0)
NCORES = 8
T = N // NCORES                # 512 tokens per core
TT = T // 128                  # 4 token tiles
C_OFF = 18.9375                # output offset: device stores out + C_OFF

E4 = ml_dtypes.float8_e4m3
E3 = ml_dtypes.float8_e3m4
BF16 = ml_dtypes.bfloat16

_COMPILED = {}


def _chunks(total, width):
    return [(s, min(width, total - s)) for s in range(0, total, width)]


def _build():
    import concourse.tile as tile
    from concourse import bacc, mybir
    from concourse.alu_op_type import AluOpType

    F32 = mybir.dt.float32
    F8E4 = mybir.dt.float8e4
    F8E3 = mybir.dt.float8e3
    Exp = mybir.ActivationFunctionType.Exp
    Ln = mybir.ActivationFunctionType.Ln
    DR = mybir.MatmulPerfMode.DoubleRow
    AX = mybir.AxisListType.X

    nc = bacc.Bacc("TRN2", target_bir_lowering=False, debug=False,
                   num_devices=NCORES)

    xT_d = nc.dram_tensor("xT", [H, T], F8E4, kind="ExternalInput").ap()
    hwT_d = nc.dram_tensor("hwT", [H, HEAD_PAD], F8E4, kind="ExternalInput").ap()
    hb_d = nc.dram_tensor("hb", [128, HEAD_PAD], F8E3, kind="ExternalInput").ap()
    w01_d = nc.dram_tensor("w01T", [H, P0], F8E4, kind="ExternalInput").ap()
    w02_d = nc.dram_tensor("w02T", [P0, V0], F8E4, kind="ExternalInput").ap()
    w11_d = nc.dram_tensor("w11T", [H, P1], F8E4, kind="ExternalInput").ap()
    w12_d = nc.dram_tensor("w12T", [P1, V1P], F8E4, kind="ExternalInput").ap()
    out_d = nc.dram_tensor("out", [T, VOCAB], F8E4, kind="ExternalOutput").ap()

    x_r = xT_d.rearrange("(k p) t -> p k t", p=128)        # [128, 8, 512]
    hw_r = hwT_d.rearrange("(k p) v -> p k v", p=128)      # [128, 8, 4096]
    w01_r = w01_d.rearrange("(k p) m -> p k m", p=128)     # [128, 8, 1024]
    w02_r = w02_d.rearrange("(k p) v -> p k v", p=128)     # [128, 8, 16000]
    w11_r = w11_d.rearrange("(k p) m -> p k m", p=128)     # [128, 8, 256]
    w12_r = w12_d.rearrange("(k p) v -> p k v", p=128)     # [128, 2, 30272]

    SUPW = 2048
    sup_h = _chunks(HEAD_PAD, 1024)        # 4 supers of 1024
    sup_t0 = _chunks(V0, SUPW)             # 8 (last 1664)
    sup_t1 = _chunks(V1P, SUPW)            # 15 (last 1600)

    def batches_of(sups, bw):
        out, cur0, curw, last = [], None, 0, None
        for si, (c0, w) in enumerate(sups):
            if cur0 is None:
                cur0 = c0
            curw += w
            last = si
            if curw >= bw:
                out.append((last, cur0, curw))
                cur0, curw = None, 0
        if curw:
            out.append((last, cur0, curw))
        return out

    bat_h = batches_of(sup_h, 4096)        # 1 batch (4096)
    bat_t0 = batches_of(sup_t0, 8192)      # 2 (8192, 7808)
    bat_t1 = batches_of(sup_t1, 8192)      # 4 (3x8192, 5696)

    with tile.TileContext(nc, pool_alloc_mode="queue") as tc:
        # pool stack: long-lived below, transients on top (LIFO releases)
        smalls = tc.alloc_tile_pool(name="smalls", bufs=1)
        stage = tc.alloc_tile_pool(name="stage", bufs=3)
        scr = tc.alloc_tile_pool(name="scr", bufs=2)
        psum_pool = tc.alloc_tile_pool(name="psum", bufs=2, space="PSUM")
        persist = tc.alloc_tile_pool(name="persist", bufs=1)
        h0T_s = persist.tile([128, 8, T], F8E4, tag="h0T")
        h1T_s = persist.tile([128, 2, T], F8E4, tag="h1T")
        t1segs = {}
        t1segpA = tc.alloc_tile_pool(name="t1segsA", bufs=1)
        for t in (0, 1):
            t1segs[t] = t1segpA.tile([128, V1P], F8E3, tag=f"t1seg{t}",
                                     name=f"t1seg{t}")
        t1wp = tc.alloc_tile_pool(name="t1wp", bufs=2)
        hsegp = tc.alloc_tile_pool(name="hsegs", bufs=1)
        hsegs = [hsegp.tile([128, HEAD_PAD], F8E3, tag=f"hseg{t}",
                            name=f"hseg{t}")
                 for t in range(TT)]

        # ---- tiny per-row scalars ----
        def sc(tag, w=1):
            return smalls.tile([128, w], F32, tag=tag, name=tag)

        zb = sc("zb")
        nc.vector.memset(zb, 0.0)
        Zt4 = sc("Zt4", TT)
        lse4 = {"h": sc("lse_h4", TT), "t0": sc("lse_t0_4", TT),
                "t1": sc("lse_t1_4", TT)}
        l4x = [sc(f"l4x{t}", 2) for t in range(TT)]        # cols 4000,4001
        dh = [sc(f"dh{t}") for t in range(TT)]
        ndh = [sc(f"ndh{t}") for t in range(TT)]
        d0 = [sc(f"d0_{t}") for t in range(TT)]
        nd0 = [sc(f"nd0_{t}") for t in range(TT)]
        d1 = [sc(f"d1_{t}") for t in range(TT)]
        nd1 = [sc(f"nd1_{t}") for t in range(TT)]
        zh_p = [sc(f"zh_p{t}", len(bat_h)) for t in range(TT)]
        z0_p = [sc(f"z0_p{t}", len(bat_t0)) for t in range(TT)]
        z1_p = [sc(f"z1_p{t}", len(bat_t1)) for t in range(TT)]

        # ---- transient pools for proj + head ----
        xtp = tc.alloc_tile_pool(name="xtp", bufs=1)
        xT_s = xtp.tile([128, 8, T], F8E4, tag="xT", name="xT")
        nc.sync.dma_start(out=xT_s, in_=x_r)
        hbp = tc.alloc_tile_pool(name="hbp", bufs=1)
        hb_s = hbp.tile([128, HEAD_PAD], F8E3, tag="hb", name="hb")
        nc.sync.dma_start(out=hb_s, in_=hb_d)
        hwp = tc.alloc_tile_pool(name="hwp", bufs=2)
        projw = tc.alloc_tile_pool(name="projw", bufs=1)
        w01_s = projw.tile([128, 8, P0], F8E4, tag="w01", name="w01")
        w11_s = projw.tile([128, 8, P1], F8E4, tag="w11", name="w11")
        nc.sync.dma_start(out=w01_s, in_=w01_r)
        nc.sync.dma_start(out=w11_s, in_=w11_r)

        # ---- phase 0: proj matmuls ----
        for half in range(2):   # h0T: 8 m-tiles, 4 per psum super
            ps = psum_pool.tile([128, 2048], F32, tag="ps", name=f"psh0{half}")
            for g in range(4):
                for m in range(4):
                    mm = half * 4 + m
                    nc.tensor.matmul(
                        ps[:, m * 512:(m + 1) * 512],
                        lhsT=w01_s[:, 2 * g:2 * g + 2, mm * 128:(mm + 1) * 128],
                        rhs=xT_s[:, 2 * g:2 * g + 2, :],
                        start=(g == 0), stop=(g == 3),
                        perf_mode=DR,
                    )
            nc.vector.tensor_copy(out=h0T_s[:, 4 * half:4 * half + 4, :],
                                  in_=ps)
        ps1 = psum_pool.tile([128, 2048], F32, tag="ps", name="ps_h1")
        for g in range(4):
            for m in range(2):
                nc.tensor.matmul(
                    ps1[:, m * 512:(m + 1) * 512],
                    lhsT=w11_s[:, 2 * g:2 * g + 2, m * 128:(m + 1) * 128],
                    rhs=xT_s[:, 2 * g:2 * g + 2, :],
                    start=(g == 0), stop=(g == 3),
                    perf_mode=DR,
                )
        nc.vector.tensor_copy(out=h1T_s, in_=ps1[:, 0:1024])
        projw.release()

        pend = []   # deferred output-unit emitters

        def drain(n):
            for _ in range(min(n, len(pend))):
                pend.pop(0)()

        def cluster_units(name, sups, bats, tts, wloader, lhsT_of, Kg, segs,
                          zp, evac_act, bias=None, l4cap=None):
            """Return list of per-(si,tt) unit closures (si-major order)."""
            bat_next = {tt: 0 for tt in tts}

            def unit(si, tt):
                c0, w = sups[si]
                wt_of = wloader(si)
                ps = psum_pool.tile([128, SUPW], F32, tag="ps",
                                    name=f"ps_{name}")
                cks = _chunks(w, 512)
                for g in range(Kg):
                    wt, off = wt_of(g)
                    for (cc, cw) in cks:
                        nc.tensor.matmul(
                            ps[:, cc:cc + cw],
                            lhsT=lhsT_of(g, tt),
                            rhs=wt[:, :, off + cc:off + cc + cw],
                            start=(g == 0), stop=(g == Kg - 1),
                            perf_mode=DR,
                        )
                seg = segs[tt]
                if l4cap is not None and si == l4cap[0]:
                    nc.vector.scalar_tensor_tensor(
                        out=l4x[tt], in0=ps[:, l4cap[1]:l4cap[1] + 2],
                        scalar=1.0, in1=hb_s[:, 4000:4002],
                        op0=AluOpType.mult, op1=AluOpType.add)
                if bias is not None:
                    nc.vector.scalar_tensor_tensor(
                        out=seg[:, c0:c0 + w], in0=ps[:, :w], scalar=1.0,
                        in1=bias[:, c0:c0 + w],
                        op0=AluOpType.mult, op1=AluOpType.add)
                elif evac_act(si, tt):
                    nc.scalar.copy(out=seg[:, c0:c0 + w], in_=ps[:, :w])
                else:
                    nc.vector.tensor_copy(out=seg[:, c0:c0 + w],
                                          in_=ps[:, :w])
                b = bat_next[tt]
                if b < len(bats) and bats[b][0] == si:
                    _, bc0, bw = bats[b]
                    ex = scr.tile([128, 8192], F8E4, tag="ex", name="ex")
                    nc.scalar.activation(
                        out=ex[:, :bw], in_=seg[:, bc0:bc0 + bw],
                        func=Exp, bias=zb, scale=1.0,
                        accum_out=zp[tt][:, b:b + 1])
                    bat_next[tt] = b + 1

            return [(lambda si=si, tt=tt: unit(si, tt))
                    for si in range(len(sups)) for tt in tts]

        def finish_rows(key, tts, zp, nbat, d_out, nd_out, extra):
            lse = lse4[key]
            for tt in tts:
                nc.vector.reduce_sum(out=Zt4[:, tt:tt + 1],
                                     in_=zp[tt][:, 0:nbat], axis=AX)
            a, b = min(tts), max(tts) + 1
            nc.scalar.activation(out=lse[:, a:b], in_=Zt4[:, a:b],
                                 func=Ln, bias=zb, scale=1.0)
            for tt in tts:
                lse_tt = lse[:, tt:tt + 1]
                if extra is None:
                    nc.vector.tensor_scalar_sub(d_out[tt], lse_tt, C_OFF)
                else:
                    ex_key, l4col = extra
                    nc.vector.tensor_add(d_out[tt], lse_tt,
                                         lse4[ex_key][:, tt:tt + 1])
                    nc.vector.scalar_tensor_tensor(
                        out=d_out[tt], in0=d_out[tt], scalar=C_OFF,
                        in1=l4x[tt][:, l4col:l4col + 1],
                        op0=AluOpType.subtract, op1=AluOpType.subtract)
                nc.vector.tensor_sub(nd_out[tt], zb, d_out[tt])

        def emit_units(segs, tt, d_ap, nd_ap, out_c0, width, on_act):
            r0 = tt * 128
            for ui, (c0, cw) in enumerate(_chunks(width, 4096)):
                def emit(c0=c0, cw=cw, ui=ui, tt=tt):
                    seg = segs[tt]
                    st = stage.tile([128, 4096], F8E4, tag="st", name="st")
                    if on_act(ui):
                        nc.scalar.add(st[:, :cw], seg[:, c0:c0 + cw], nd_ap)
                    else:
                        nc.vector.tensor_scalar_sub(
                            st[:, :cw], seg[:, c0:c0 + cw], d_ap)
                    nc.sync.dma_start(
                        out=out_d[r0:r0 + 128, out_c0 + c0:out_c0 + c0 + cw],
                        in_=st[:, :cw])
                pend.append(emit)

        # ================= HEAD (alone; lse_h gates all tail outputs) ====
        hw_tiles = {}

        def hw_loader(si):
            if si not in hw_tiles:
                t_ = hwp.tile([128, 8, 1024], F8E4, tag="hw", name=f"hw{si}")
                nc.sync.dma_start(
                    out=t_, in_=hw_r[:, :, si * 1024:(si + 1) * 1024])
                hw_tiles[si] = t_
            tile_ = hw_tiles[si]
            return lambda g: (tile_[:, 2 * g:2 * g + 2, :], 0)

        for u in cluster_units("h", sup_h, bat_h, range(TT), hw_loader,
                               lambda g, tt: xT_s[:, 2 * g:2 * g + 2,
                                                  tt * 128:(tt + 1) * 128],
                               4, hsegs, zh_p, lambda si, tt: False,
                               bias=hb_s, l4cap=(3, 928)):
            u()
        finish_rows("h", range(TT), zh_p, len(bat_h), dh, ndh, None)
        for tt in range(TT):
            emit_units(hsegs, tt, dh[tt], ndh[tt], 0, CUT0,
                       lambda ui: False)
        hwp.release()
        hbp.release()
        xtp.release()

        # ================= T0 pools =================
        t0segp = tc.alloc_tile_pool(name="t0segs", bufs=1)
        t0segs = [t0segp.tile([128, V0], F8E3, tag=f"t0seg{t}",
                              name=f"t0seg{t}")
                  for t in range(TT)]
        t0wp = tc.alloc_tile_pool(name="t0wp", bufs=3)
        t0_tiles = {}

        def t0_loader(si):
            if si not in t0_tiles:
                c0, w = sup_t0[si]
                ts = []
                for kh in range(2):
                    t_ = t0wp.tile([128, 4, SUPW], F8E4, tag="w02",
                                   name=f"w02_{si}_{kh}")
                    nc.sync.dma_start(
                        out=t_[:, :, :w],
                        in_=w02_r[:, 4 * kh:4 * kh + 4, c0:c0 + w])
                    ts.append(t_)
                t0_tiles[si] = ts
            ts = t0_tiles[si]
            return lambda g: (ts[g // 2][:, 2 * (g % 2):2 * (g % 2) + 2, :], 0)

        def t1_loader_mk(tag):
            t1_tiles = {}

            def t1_loader(si):
                if si not in t1_tiles:
                    c0, w = sup_t1[si]
                    t_ = t1wp.tile([128, 2, SUPW], F8E4, tag="w12",
                                   name=f"w12_{tag}_{si}")
                    nc.sync.dma_start(out=t_[:, :, :w],
                                      in_=w12_r[:, :, c0:c0 + w])
                    t1_tiles[si] = t_
                tile_ = t1_tiles[si]
                return lambda g: (tile_, 0)
            return t1_loader

        lhsT_t0 = lambda g, tt: h0T_s[:, 2 * g:2 * g + 2,
                                      tt * 128:(tt + 1) * 128]
        lhsT_t1 = lambda g, tt: h1T_s[:, 0:2, tt * 128:(tt + 1) * 128]

        # ========== P2: interleave t0 (all tt) with t1 group A (tt0,1) ====
        u_t0 = cluster_units("t0", sup_t0, bat_t0, range(TT), t0_loader,
                             lhsT_t0, 4, t0segs, z0_p,
                             lambda si, tt: si % 4 == 1)
        u_gA = cluster_units("t1gA", sup_t1, bat_t1, [0, 1],
                             t1_loader_mk("A"), lhsT_t1, 1, t1segs, z1_p,
                             lambda si, tt: si % 2 == 1)
        # rounds: r<8 -> 4 t0 units + 2 gA units; r>=8 -> 2 gA units
        acc = [0.0]

        def pace(dpu):
            acc[0] += dpu
            k = int(acc[0])
            if k:
                acc[0] -= k
                drain(k)

        for r in range(15):
            if r < 8:
                for j in range(4):
                    u_t0[4 * r + j]()
                    pace(0.12)          # head's 4 units over t0's 32
            for j in range(2):
                u_gA[2 * r + j]()
                if r >= 8:
                    pace(1.2)           # t0's 16 units over rounds 8-14
            if r == 7:
                finish_rows("t0", range(TT), z0_p, len(bat_t0), d0, nd0,
                            ("h", 0))
                for tt in range(TT):
                    emit_units(t0segs, tt, d0[tt], nd0[tt], CUT0, V0,
                               lambda ui: False)
        finish_rows("t1", [0, 1], z1_p, len(bat_t1), d1, nd1, ("h", 1))
        for tt in (0, 1):
            emit_units(t1segs, tt, d1[tt], nd1[tt], CUT1, V1,
                       lambda ui: False)
        t0wp.release()
        t0segp.release()
        hsegp.release()

        # ========== P3: t1 group B, tt-major (tt2 pass then tt3 pass) ====
        t1segpB = tc.alloc_tile_pool(name="t1segsB", bufs=1)
        for t in (2, 3):
            t1segs[t] = t1segpB.tile([128, V1P], F8E3, tag=f"t1seg{t}",
                                     name=f"t1seg{t}")
        for tt in (2, 3):
            units = cluster_units(f"t1g{tt}", sup_t1, bat_t1, [tt],
                                  t1_loader_mk(f"B{tt}"), lhsT_t1, 1,
                                  t1segs, z1_p,
                                  lambda si, t_: si % 2 == 1)
            # tt2 pass drains gA's 16 units; tt3 pass drains tt2's 8
            dpu = 1.1 if tt == 2 else 0.6
            for u in units:
                u()
                pace(dpu)
            finish_rows("t1", [tt], z1_p, len(bat_t1), d1, nd1, ("h", 1))
            emit_units(t1segs, tt, d1[tt], nd1[tt], CUT1, V1,
                       (lambda ui: False) if tt == 2
                       else (lambda ui: ui in (1, 4, 6)))
        for u in pend:
            u()
        for p in (t1segpB, t1wp, t1segpA, persist, psum_pool, scr, stage,
                  smalls):
            p.release()

    nc.compile()
    return nc


def _get_nc():
    if "nc" not in _COMPILED:
        _COMPILED["nc"] = _build()
    return _COMPILED["nc"]


def _prep_inputs(x, head_w, head_b, t0_w1, t0_w2, t1_w1, t1_w2):
    f32 = np.float32

    hwT = np.zeros((H, HEAD_PAD), dtype=f32)
    hwT[:, :HEAD_OUT] = np.asarray(head_w, f32).T
    hb = np.full((HEAD_PAD,), -30.0, dtype=f32)
    hb[:HEAD_OUT] = np.asarray(head_b, f32)
    hbrep = np.ascontiguousarray(
        np.broadcast_to(hb, (128, HEAD_PAD))).astype(E3)

    w12T = np.zeros((P1, V1P), dtype=f32)
    w12T[:, :V1] = np.asarray(t1_w2, f32).T

    ins_common = {
        "hwT": hwT.astype(E4),
        "hb": hbrep,
        "w01T": np.ascontiguousarray(np.asarray(t0_w1, f32).T).astype(E4),
        "w02T": np.ascontiguousarray(np.asarray(t0_w2, f32).T).astype(E4),
        "w11T": np.ascontiguousarray(np.asarray(t1_w1, f32).T).astype(E4),
        "w12T": w12T.astype(E4),
    }
    in_maps = []
    for c in range(NCORES):
        xs = np.asarray(x[c * T:(c + 1) * T], f32)
        m = {"xT": np.ascontiguousarray(xs.T).astype(E4)}
        m.update(ins_common)
        in_maps.append(m)
    return in_maps


def run(trace=False, **inputs):
    from concourse.bass_utils import run_bass_kernel_spmd

    if trace:
        try:
            if "antenv.axon_hooks" not in sys.modules:
                if "/root/.axon_site" not in sys.path:
                    sys.path.append("/root/.axon_site")
                import trn_agent_boot.trn_boot as tb
                hook = tb._ntff_profile_via_ctypes("/opt/axon/libaxon_pjrt.so")
                mod = types.ModuleType("antenv.axon_hooks")
                mod.get_axon_ntff_profile_hook = lambda: hook
                sys.modules["antenv.axon_hooks"] = mod
        except Exception:
            trace = False

    nc = _get_nc()
    in_maps = _prep_inputs(**inputs)
    last_err = None
    for attempt in range(3):
        try:
            res = run_bass_kernel_spmd(nc, in_maps,
                                       core_ids=list(range(NCORES)),
                                       trace=trace)
            break
        except Exception as e:  # transient NRT device errors: retry
            last_err = e
    else:
        raise last_err
    out = np.concatenate(
        [res.results[i]["out"].astype(np.float32) for i in range(NCORES)],
        axis=0)
    out -= C_OFF
    return out, res


def kernel(**inputs):
    out, _ = run(trace=False, **inputs)
    return out


if __name__ == "__main__":
    rng = np.random.default_rng(0)
    ins = {
        "x": rng.standard_normal((N, H), dtype=np.float32),
        "head_w": (rng.standard_normal((HEAD_OUT, H), dtype=np.float32) / 32),
        "head_b": (rng.standard_normal(HEAD_OUT).astype(np.float32) * 0.01),
        "t0_w1": (rng.standard_normal((P0, H), dtype=np.float32) / 32),
        "t0_w2": (rng.standard_normal((CUT1 - CUT0, P0), dtype=np.float32) / 32),
        "t1_w1": (rng.standard_normal((P1, H), dtype=np.float32) / 32),
        "t1_w2": (rng.standard_normal((VOCAB - CUT1, P1), dtype=np.float32) / 16),
    }
    out, res = run(trace=False, **ins)
    print("out", out.shape, out.dtype)


# revision 17
# speedup vs baseline: 1.3831x; 1.0712x over previous
"""Adaptive softmax (head + 2 factorized tails) on 8 TRN2 NeuronCores.

v2: fp8 everywhere. Data-parallel over tokens (512/core), weights
replicated, cast to fp8-e4m3 host-side. All matmuls run DoubleRow
(2 k-tiles per instruction, 2x bf16 FLOP rate), with g-major/chunk-minor
accumulation so LDWEIGHTS amortizes over 4 matmuls. Logit segments are
staged in fp8-e3m4 SBUF tiles; PSUM evacuation is split between DVE
(tensor_copy / bias-adding scalar_tensor_tensor) and ACT (Identity
activation) per a measured-rate balance; Exp+row-sum partials run on ACT
reading PSUM directly. Output units compute (logit - d + C) into
fp8-e4m3 staging tiles (DVE tensor_scalar at the 2x fp8 rate, plus ACT
Identity+bias for the tail-most units) and DMA out; the host decodes
with float32(out) - C. Head bias (+ -30 padding) is folded in as a
free-dim bias tile added during head evacuation. Cluster order:
proj -> head -> t0 -> t1(tt0-2) -> t1(tt3); each cluster's output units
drain interleaved into the next cluster's compute so the serial tail is
only tt3's vocab slice.
"""

import sys
import types

for _p in ("/opt/trn_rl_repo",):
    if _p not in sys.path:
        sys.path.append(_p)

import numpy as np
import ml_dtypes

N, H = 4096, 1024
CUT0, CUT1, VOCAB = 4000, 20000, 50257
HEAD_OUT = CUT0 + 2            # 4002
HEAD_PAD = 4096                # padded head cols (pad logit = -30 via bias)
P0, P1 = 1024, 256
V0 = CUT1 - CUT0               # 16000
V1 = VOCAB - CUT1              # 30257
V1P = 30272                    # padded (mult of 64; pad weight cols = 0)
NCORES = 8
T = N // NCORES                # 512 tokens per core
TT = T // 128                  # 4 token tiles
C_OFF = 18.9375                # output offset: device stores out + C_OFF

E4 = ml_dtypes.float8_e4m3
E3 = ml_dtypes.float8_e3m4
BF16 = ml_dtypes.bfloat16

_COMPILED = {}


def _chunks(total, width):
    return [(s, min(width, total - s)) for s in range(0, total, width)]


def _build():
    import concourse.tile as tile
    from concourse import bacc, mybir
    from concourse.alu_op_type import AluOpType

    F32 = mybir.dt.float32
    F8E4 = mybir.dt.float8e4
    F8E3 = mybir.dt.float8e3
    Exp = mybir.ActivationFunctionType.Exp
    Ln = mybir.ActivationFunctionType.Ln
    DR = mybir.MatmulPerfMode.DoubleRow
    AX = mybir.AxisListType.X

    nc = bacc.Bacc("TRN2", target_bir_lowering=False, debug=False,
                   num_devices=NCORES)

    xT_d = nc.dram_tensor("xT", [H, T], F8E4, kind="ExternalInput").ap()
    hwT_d = nc.dram_tensor("hwT", [H, HEAD_PAD], F8E4, kind="ExternalInput").ap()
    hb_d = nc.dram_tensor("hb", [128, HEAD_PAD], F8E3, kind="ExternalInput").ap()
    w01_d = nc.dram_tensor("w01T", [H, P0], F8E4, kind="ExternalInput").ap()
    w02_d = nc.dram_tensor("w02T", [P0, V0], F8E4, kind="ExternalInput").ap()
    w11_d = nc.dram_tensor("w11T", [H, P1], F8E4, kind="ExternalInput").ap()
    w12_d = nc.dram_tensor("w12T", [P1, V1P], F8E4, kind="ExternalInput").ap()
    out_d = nc.dram_tensor("out", [T, VOCAB], F8E4, kind="ExternalOutput").ap()

    x_r = xT_d.rearrange("(k p) t -> p k t", p=128)        # [128, 8, 512]
    hw_r = hwT_d.rearrange("(k p) v -> p k v", p=128)      # [128, 8, 4096]
    w01_r = w01_d.rearrange("(k p) m -> p k m", p=128)     # [128, 8, 1024]
    w02_r = w02_d.rearrange("(k p) v -> p k v", p=128)     # [128, 8, 16000]
    w11_r = w11_d.rearrange("(k p) m -> p k m", p=128)     # [128, 8, 256]
    w12_r = w12_d.rearrange("(k p) v -> p k v", p=128)     # [128, 2, 30272]

    SUPW = 2048
    sup_h = _chunks(HEAD_PAD, 1024)        # 4 supers of 1024
    sup_t0 = _chunks(V0, SUPW)             # 8 (last 1664)
    sup_t1 = _chunks(V1P, SUPW)            # 15 (last 1600)

    def batches_of(sups, par):
        """per-tt staggered exp batches: boundaries at si%2==par plus the
        final super; returns [(last_si, col0, width)]"""
        S = len(sups)
        ends = [si for si in range(S) if si % 2 == par or si == S - 1]
        out, prev = [], 0
        for e in ends:
            c0 = sups[prev][0]
            w = sups[e][0] + sups[e][1] - c0
            out.append((e, c0, w))
            prev = e + 1
        return out

    bat_h = {tt: batches_of(sup_h, 1) for tt in range(TT)}   # 2 per tt
    bat_t0 = {tt: batches_of(sup_t0, tt % 2) for tt in range(TT)}
    bat_t1 = {tt: batches_of(sup_t1, tt % 2) for tt in range(TT)}

    with tile.TileContext(nc, pool_alloc_mode="queue") as tc:
        # pool stack: long-lived below, transients on top (LIFO releases)
        smalls = tc.alloc_tile_pool(name="smalls", bufs=1)
        stage = tc.alloc_tile_pool(name="stage", bufs=3)
        scr = tc.alloc_tile_pool(name="scr", bufs=2)
        psum_pool = tc.alloc_tile_pool(name="psum", bufs=2, space="PSUM")
        persist = tc.alloc_tile_pool(name="persist", bufs=1)
        h0T_s = persist.tile([128, 8, T], F8E4, tag="h0T")
        h1T_s = persist.tile([128, 2, T], F8E4, tag="h1T")
        t1segs = {}
        t1segpA = tc.alloc_tile_pool(name="t1segsA", bufs=1)
        for t in (0, 1):
            t1segs[t] = t1segpA.tile([128, V1P], F8E3, tag=f"t1seg{t}",
                                     name=f"t1seg{t}")
        t1wp = tc.alloc_tile_pool(name="t1wp", bufs=2)
        hsegp = tc.alloc_tile_pool(name="hsegs", bufs=1)
        hsegs = [hsegp.tile([128, HEAD_PAD], F8E3, tag=f"hseg{t}",
                            name=f"hseg{t}")
                 for t in range(TT)]

        # ---- tiny per-row scalars ----
        def sc(tag, w=1):
            return smalls.tile([128, w], F32, tag=tag, name=tag)

        zb = sc("zb")
        nc.vector.memset(zb, 0.0)
        Zt4 = sc("Zt4", TT)
        lse4 = {"h": sc("lse_h4", TT), "t0": sc("lse_t0_4", TT),
                "t1": sc("lse_t1_4", TT)}
        l4x = [sc(f"l4x{t}", 2) for t in range(TT)]        # cols 4000,4001
        dh = [sc(f"dh{t}") for t in range(TT)]
        ndh = [sc(f"ndh{t}") for t in range(TT)]
        d0 = [sc(f"d0_{t}") for t in range(TT)]
        nd0 = [sc(f"nd0_{t}") for t in range(TT)]
        d1 = [sc(f"d1_{t}") for t in range(TT)]
        nd1 = [sc(f"nd1_{t}") for t in range(TT)]
        zh_p = [sc(f"zh_p{t}", len(bat_h[t])) for t in range(TT)]
        z0_p = [sc(f"z0_p{t}", len(bat_t0[t])) for t in range(TT)]
        z1_p = [sc(f"z1_p{t}", len(bat_t1[t])) for t in range(TT)]

        # ---- transient pools for proj + head ----
        xtp = tc.alloc_tile_pool(name="xtp", bufs=1)
        xT_s = xtp.tile([128, 8, T], F8E4, tag="xT", name="xT")
        nc.sync.dma_start(out=xT_s, in_=x_r)
        hbp = tc.alloc_tile_pool(name="hbp", bufs=1)
        hb_s = hbp.tile([128, HEAD_PAD], F8E3, tag="hb", name="hb")
        nc.sync.dma_start(out=hb_s, in_=hb_d)
        hwp = tc.alloc_tile_pool(name="hwp", bufs=2)
        projw = tc.alloc_tile_pool(name="projw", bufs=1)
        w01_s = projw.tile([128, 8, P0], F8E4, tag="w01", name="w01")
        w11_s = projw.tile([128, 8, P1], F8E4, tag="w11", name="w11")
        nc.sync.dma_start(out=w01_s, in_=w01_r)
        nc.sync.dma_start(out=w11_s, in_=w11_r)

        # ---- phase 0: proj matmuls ----
        for half in range(2):   # h0T: 8 m-tiles, 4 per psum super
            ps = psum_pool.tile([128, 2048], F32, tag="ps", name=f"psh0{half}")
            for g in range(4):
                for m in range(4):
                    mm = half * 4 + m
                    nc.tensor.matmul(
                        ps[:, m * 512:(m + 1) * 512],
                        lhsT=w01_s[:, 2 * g:2 * g + 2, mm * 128:(mm + 1) * 128],
                        rhs=xT_s[:, 2 * g:2 * g + 2, :],
                        start=(g == 0), stop=(g == 3),
                        perf_mode=DR,
                    )
            nc.vector.tensor_copy(out=h0T_s[:, 4 * half:4 * half + 4, :],
                                  in_=ps)
        ps1 = psum_pool.tile([128, 2048], F32, tag="ps", name="ps_h1")
        for g in range(4):
            for m in range(2):
                nc.tensor.matmul(
                    ps1[:, m * 512:(m + 1) * 512],
                    lhsT=w11_s[:, 2 * g:2 * g + 2, m * 128:(m + 1) * 128],
                    rhs=xT_s[:, 2 * g:2 * g + 2, :],
                    start=(g == 0), stop=(g == 3),
                    perf_mode=DR,
                )
        nc.vector.tensor_copy(out=h1T_s, in_=ps1[:, 0:1024])
        projw.release()

        pend = []   # deferred output-unit emitters

        def drain(n):
            for _ in range(min(n, len(pend))):
                pend.pop(0)()

        def cluster_units(name, sups, bats, tts, wloader, lhsT_of, Kg, segs,
                          zp, evac_act, bias=None, l4cap=None):
            """Return list of per-(si,tt) unit closures (si-major order)."""
            bat_next = {tt: 0 for tt in tts}

            def unit(si, tt):
                c0, w = sups[si]
                wt_of = wloader(si)
                ps = psum_pool.tile([128, SUPW], F32, tag="ps",
                                    name=f"ps_{name}")
                cks = _chunks(w, 512)
                for g in range(Kg):
                    wt, off = wt_of(g)
                    for (cc, cw) in cks:
                        nc.tensor.matmul(
                            ps[:, cc:cc + cw],
                            lhsT=lhsT_of(g, tt),
                            rhs=wt[:, :, off + cc:off + cc + cw],
                            start=(g == 0), stop=(g == Kg - 1),
                            perf_mode=DR,
                        )
                seg = segs[tt]
                if l4cap is not None and si == l4cap[0]:
                    nc.vector.scalar_tensor_tensor(
                        out=l4x[tt], in0=ps[:, l4cap[1]:l4cap[1] + 2],
                        scalar=1.0, in1=hb_s[:, 4000:4002],
                        op0=AluOpType.mult, op1=AluOpType.add)
                if bias is not None:
                    nc.vector.scalar_tensor_tensor(
                        out=seg[:, c0:c0 + w], in0=ps[:, :w], scalar=1.0,
                        in1=bias[:, c0:c0 + w],
                        op0=AluOpType.mult, op1=AluOpType.add)
                elif evac_act(si, tt):
                    nc.scalar.copy(out=seg[:, c0:c0 + w], in_=ps[:, :w])
                else:
                    nc.vector.tensor_copy(out=seg[:, c0:c0 + w],
                                          in_=ps[:, :w])
                bl = bats[tt]
                b = bat_next[tt]
                if b < len(bl) and bl[b][0] == si:
                    _, bc0, bw = bl[b]
                    ex = scr.tile([128, 4096], F8E4, tag="ex", name="ex")
                    nc.scalar.activation(
                        out=ex[:, :bw], in_=seg[:, bc0:bc0 + bw],
                        func=Exp, bias=zb, scale=1.0,
                        accum_out=zp[tt][:, b:b + 1])
                    bat_next[tt] = b + 1

            return [(lambda si=si, tt=tt: unit(si, tt))
                    for si in range(len(sups)) for tt in tts]

        def finish_rows(key, tts, zp, bats, d_out, nd_out, extra):
            lse = lse4[key]
            for tt in tts:
                nc.vector.reduce_sum(out=Zt4[:, tt:tt + 1],
                                     in_=zp[tt][:, 0:len(bats[tt])], axis=AX)
            a, b = min(tts), max(tts) + 1
            nc.scalar.activation(out=lse[:, a:b], in_=Zt4[:, a:b],
                                 func=Ln, bias=zb, scale=1.0)
            for tt in tts:
                lse_tt = lse[:, tt:tt + 1]
                if extra is None:
                    nc.vector.tensor_scalar_sub(d_out[tt], lse_tt, C_OFF)
                else:
                    ex_key, l4col = extra
                    nc.vector.tensor_add(d_out[tt], lse_tt,
                                         lse4[ex_key][:, tt:tt + 1])
                    nc.vector.scalar_tensor_tensor(
                        out=d_out[tt], in0=d_out[tt], scalar=C_OFF,
                        in1=l4x[tt][:, l4col:l4col + 1],
                        op0=AluOpType.subtract, op1=AluOpType.subtract)
                nc.vector.tensor_sub(nd_out[tt], zb, d_out[tt])

        def emit_units(segs, tt, d_ap, nd_ap, out_c0, width, on_act):
            r0 = tt * 128
            for ui, (c0, cw) in enumerate(_chunks(width, 4096)):
                def emit(c0=c0, cw=cw, ui=ui, tt=tt):
                    seg = segs[tt]
                    st = stage.tile([128, 4096], F8E4, tag="st", name="st")
                    if on_act(ui):
                        nc.scalar.add(st[:, :cw], seg[:, c0:c0 + cw], nd_ap)
                    else:
                        nc.vector.tensor_scalar_sub(
                            st[:, :cw], seg[:, c0:c0 + cw], d_ap)
                    nc.sync.dma_start(
                        out=out_d[r0:r0 + 128, out_c0 + c0:out_c0 + c0 + cw],
                        in_=st[:, :cw])
                pend.append(emit)

        # ================= HEAD (alone; lse_h gates all tail outputs) ====
        hw_tiles = {}

        def hw_loader(si):
            if si not in hw_tiles:
                t_ = hwp.tile([128, 8, 1024], F8E4, tag="hw", name=f"hw{si}")
                nc.sync.dma_start(
                    out=t_, in_=hw_r[:, :, si * 1024:(si + 1) * 1024])
                hw_tiles[si] = t_
            tile_ = hw_tiles[si]
            return lambda g: (tile_[:, 2 * g:2 * g + 2, :], 0)

        for u in cluster_units("h", sup_h, bat_h, range(TT), hw_loader,
                               lambda g, tt: xT_s[:, 2 * g:2 * g + 2,
                                                  tt * 128:(tt + 1) * 128],
                               4, hsegs, zh_p, lambda si, tt: False,
                               bias=hb_s, l4cap=(3, 928)):
            u()
        finish_rows("h", range(TT), zh_p, bat_h, dh, ndh, None)
        for tt in range(TT):
            emit_units(hsegs, tt, dh[tt], ndh[tt], 0, CUT0,
                       lambda ui: False)
        hwp.release()
        hbp.release()
        xtp.release()

        # ================= T0 pools =================
        t0segp = tc.alloc_tile_pool(name="t0segs", bufs=1)
        t0segs = [t0segp.tile([128, V0], F8E3, tag=f"t0seg{t}",
                              name=f"t0seg{t}")
                  for t in range(TT)]
        t0wp = tc.alloc_tile_pool(name="t0wp", bufs=3)
        t0_tiles = {}

        def t0_loader(si):
            if si not in t0_tiles:
                c0, w = sup_t0[si]
                ts = []
                for kh in range(2):
                    t_ = t0wp.tile([128, 4, SUPW], F8E4, tag="w02",
                                   name=f"w02_{si}_{kh}")
                    nc.sync.dma_start(
                        out=t_[:, :, :w],
                        in_=w02_r[:, 4 * kh:4 * kh + 4, c0:c0 + w])
                    ts.append(t_)
                t0_tiles[si] = ts
            ts = t0_tiles[si]
            return lambda g: (ts[g // 2][:, 2 * (g % 2):2 * (g % 2) + 2, :], 0)

        def t1_loader_mk(tag):
            t1_tiles = {}

            def t1_loader(si):
                if si not in t1_tiles:
                    c0, w = sup_t1[si]
                    t_ = t1wp.tile([128, 2, SUPW], F8E4, tag="w12",
                                   name=f"w12_{tag}_{si}")
                    nc.sync.dma_start(out=t_[:, :, :w],
                                      in_=w12_r[:, :, c0:c0 + w])
                    t1_tiles[si] = t_
                tile_ = t1_tiles[si]
                return lambda g: (tile_, 0)
            return t1_loader

        lhsT_t0 = lambda g, tt: h0T_s[:, 2 * g:2 * g + 2,
                                      tt * 128:(tt + 1) * 128]
        lhsT_t1 = lambda g, tt: h1T_s[:, 0:2, tt * 128:(tt + 1) * 128]

        # ========== P2: interleave t0 (all tt) with t1 group A (tt0,1) ====
        u_t0 = cluster_units("t0", sup_t0, bat_t0, range(TT), t0_loader,
                             lhsT_t0, 4, t0segs, z0_p,
                             lambda si, tt: False)
        u_gA = cluster_units("t1gA", sup_t1, bat_t1, [0, 1],
                             t1_loader_mk("A"), lhsT_t1, 1, t1segs, z1_p,
                             lambda si, tt: False)
        # rounds: r<8 -> 4 t0 units + 2 gA units; r>=8 -> 2 gA units
        acc = [0.0]

        def pace(dpu):
            acc[0] += dpu
            k = int(acc[0])
            if k:
                acc[0] -= k
                drain(k)

        for r in range(15):
            if r < 8:
                for j in range(4):
                    u_t0[4 * r + j]()
                    pace(0.12)          # head's 4 units over t0's 32
            for j in range(2):
                u_gA[2 * r + j]()
                if r >= 8:
                    pace(1.2)           # t0's 16 units over rounds 8-14
            if r == 7:
                finish_rows("t0", range(TT), z0_p, bat_t0, d0, nd0,
                            ("h", 0))
                for tt in range(TT):
                    emit_units(t0segs, tt, d0[tt], nd0[tt], CUT0, V0,
                               lambda ui: False)
        finish_rows("t1", [0, 1], z1_p, bat_t1, d1, nd1, ("h", 1))
        for tt in (0, 1):
            emit_units(t1segs, tt, d1[tt], nd1[tt], CUT1, V1,
                       lambda ui: False)
        t0wp.release()
        t0segp.release()
        hsegp.release()

        # ========== P3: t1 group B, tt-major (tt2 pass then tt3 pass) ====
        t1segpB = tc.alloc_tile_pool(name="t1segsB", bufs=1)
        for t in (2, 3):
            t1segs[t] = t1segpB.tile([128, V1P], F8E3, tag=f"t1seg{t}",
                                     name=f"t1seg{t}")
        for tt in (2, 3):
            units = cluster_units(f"t1g{tt}", sup_t1, bat_t1, [tt],
                                  t1_loader_mk(f"B{tt}"), lhsT_t1, 1,
                                  t1segs, z1_p,
                                  lambda si, t_: si % 2 == 1)
            # tt2 pass drains gA's 16 units; tt3 pass drains tt2's 8
            dpu = 1.1 if tt == 2 else 0.6
            for u in units:
                u()
                pace(dpu)
            finish_rows("t1", [tt], z1_p, bat_t1, d1, nd1, ("h", 1))
            emit_units(t1segs, tt, d1[tt], nd1[tt], CUT1, V1,
                       (lambda ui: False) if tt == 2
                       else (lambda ui: ui in (1, 4, 6)))
        for u in pend:
            u()
        for p in (t1segpB, t1wp, t1segpA, persist, psum_pool, scr, stage,
                  smalls):
            p.release()

    nc.compile()
    return nc


def _get_nc():
    if "nc" not in _COMPILED:
        _COMPILED["nc"] = _build()
    return _COMPILED["nc"]


def _prep_inputs(x, head_w, head_b, t0_w1, t0_w2, t1_w1, t1_w2):
    f32 = np.float32

    hwT = np.zeros((H, HEAD_PAD), dtype=f32)
    hwT[:, :HEAD_OUT] = np.asarray(head_w, f32).T
    hb = np.full((HEAD_PAD,), -30.0, dtype=f32)
    hb[:HEAD_OUT] = np.asarray(head_b, f32)
    hbrep = np.ascontiguousarray(
        np.broadcast_to(hb, (128, HEAD_PAD))).astype(E3)

    w12T = np.zeros((P1, V1P), dtype=f32)
    w12T[:, :V1] = np.asarray(t1_w2, f32).T

    ins_common = {
        "hwT": hwT.astype(E4),
        "hb": hbrep,
        "w01T": np.ascontiguousarray(np.asarray(t0_w1, f32).T).astype(E4),
        "w02T": np.ascontiguousarray(np.asarray(t0_w2, f32).T).astype(E4),
        "w11T": np.ascontiguousarray(np.asarray(t1_w1, f32).T).astype(E4),
        "w12T": w12T.astype(E4),
    }
    in_maps = []
    for c in range(NCORES):
        xs = np.asarray(x[c * T:(c + 1) * T], f32)
        m = {"xT": np.ascontiguousarray(xs.T).astype(E4)}
        m.update(ins_common)
        in_maps.append(m)
    return in_maps


def run(trace=False, **inputs):
    from concourse.bass_utils import run_bass_kernel_spmd

    if trace:
        try:
            if "antenv.axon_hooks" not in sys.modules:
                if "/root/.axon_site" not in sys.path:
                    sys.path.append("/root/.axon_site")
                import trn_agent_boot.trn_boot as tb
                hook = tb._ntff_profile_via_ctypes("/opt/axon/libaxon_pjrt.so")
                mod = types.ModuleType("antenv.axon_hooks")
                mod.get_axon_ntff_profile_hook = lambda: hook
                sys.modules["antenv.axon_hooks"] = mod
        except Exception:
            trace = False

    nc = _get_nc()
    in_maps = _prep_inputs(**inputs)
    last_err = None
    for attempt in range(3):
        try:
            res = run_bass_kernel_spmd(nc, in_maps,
                                       core_ids=list(range(NCORES)),
                                       trace=trace)
            break
        except Exception as e:  # transient NRT device errors: retry
            last_err = e
    else:
        raise last_err
    out = np.concatenate(
        [res.results[i]["out"].astype(np.float32) for i in range(NCORES)],
        axis=0)
    out -= C_OFF
    return out, res


def kernel(**inputs):
    out, _ = run(trace=False, **inputs)
    return out


if __name__ == "__main__":
    rng = np.random.default_rng(0)
    ins = {
        "x": rng.standard_normal((N, H), dtype=np.float32),
        "head_w": (rng.standard_normal((HEAD_OUT, H), dtype=np.float32) / 32),
        "head_b": (rng.standard_normal(HEAD_OUT).astype(np.float32) * 0.01),
        "t0_w1": (rng.standard_normal((P0, H), dtype=np.float32) / 32),
        "t0_w2": (rng.standard_normal((CUT1 - CUT0, P0), dtype=np.float32) / 32),
        "t1_w1": (rng.standard_normal((P1, H), dtype=np.float32) / 32),
        "t1_w2": (rng.standard_normal((VOCAB - CUT1, P1), dtype=np.float32) / 16),
    }
    out, res = run(trace=False, **ins)
    print("out", out.shape, out.dtype)
